# revision 1
# baseline (speedup 1.0000x reference)
"""CapsNet-CIFAR100 forward pass on 8 Trainium2 NeuronCores.

Data-parallel over batch (8 images/core). Conv stem + primary caps as
matmuls (f32r / bf16 fast-path streaming); dynamic routing with every
26M-element u_hat pass produced and consumed at bf16:

  pass 0: s0 = sum_i u_hat via dense-u matmuls (u_hat never formed),
          wr streamed bf16.
  pass 1/2: software-pipelined groups of G=8 chunks.  Per iteration gi:
      exp(gi-1) [ACT] -> softmax tail of gi-1 (z, 1/z, zsel, p16) [DVE+Pool]
      -> i-sum s-matmuls of gi-2 [PE] -> produce group gi (wr DMA, block-diag
      bf16 matmuls -> PSUM halves, ACT exits to bf16 SBUF, d-major [(d,o)])
      -> b-logit head of gi (dm = uh*v, fold tree in place) [DVE+Pool].
  The 1/z softmax normalizer is folded into the i-sum matmul's selector
  operand (zsel), so c never materializes.

Capsule chunking: chunk cb in 0..127, H=cb//64, r=cb%64; the chunk's 16
capsules are (cp in {0,1}, oh in 0..7) with ch = 128H+64cp+r, i = ch*8+oh,
vector dim k = ow. Partition index within chunk: p = cp*64 + oh*8 + ow.
conv2 runs "transposed" (output partitions = (b%2, oh, ow), free = co) so
the u gather is 32 contiguous [64,64] SBUF DMAs.
"""

from contextlib import ExitStack

import numpy as np
import ml_dtypes
import concourse.bass as bass
import concourse.mybir as mybir
import concourse.tile as tile
from concourse import bacc
from concourse import bass_utils

F32 = mybir.dt.float32
F32R = mybir.dt.float32r
F16 = mybir.dt.float16
AF = mybir.ActivationFunctionType
ALU = mybir.AluOpType
AX = mybir.AxisListType

N_CORES = 8
B = 8            # batch per core
G = 8            # routing chunks per consumer group
OSPL = 84        # o-split: [0:OSPL] on DVE, [OSPL:100] on GpSimd
EPS = 1e-8

_CACHE = {}


def _build():
    nc = bacc.Bacc("TRN2", target_bir_lowering=False, debug=False,
                   num_devices=N_CORES)

    xd = nc.dram_tensor("x_sh", [B, 3, 32, 32], F32, kind="ExternalInput").ap()
    w1d = nc.dram_tensor("w1t", [9, 27, 256], F16, kind="ExternalInput").ap()
    cbd = nc.dram_tensor("cb", [256, 1], F32, kind="ExternalInput").ap()
    w2d = nc.dram_tensor("w2t", [2, 128, 81, 256], F16, kind="ExternalInput").ap()
    pbd = nc.dram_tensor("pb", [1, 256], F32, kind="ExternalInput").ap()
    wrd = nc.dram_tensor("wr", [128, 128, 1600], F16, kind="ExternalInput").ap()
    mkd = nc.dram_tensor("mask", [128, 16, 8], F16, kind="ExternalInput").ap()
    seld = nc.dram_tensor("sel", [128, 8], F16, kind="ExternalInput").ap()
    gd = nc.dram_tensor("gmat", [128, 16], F16, kind="ExternalInput").ap()
    fdram = nc.dram_tensor("fscratch", [4, 16, 256], F32, kind="Internal").ap()
    vdram = nc.dram_tensor("vscratch", [8, 1600], F16, kind="Internal").ap()
    vout = nc.dram_tensor("v_out", [B, 100, 16], F32, kind="ExternalOutput").ap()

    with tile.TileContext(nc) as tc:
        with ExitStack() as stack:
            cpool = stack.enter_context(tc.tile_pool(name="consts", bufs=1))
            rpool = stack.enter_context(tc.tile_pool(name="rconsts", bufs=1))

            # ---------- shared constants ----------
            w1sb = cpool.tile([27, 9, 256], F16, name="w1sb")
            nc.sync.dma_start(out=w1sb, in_=w1d.rearrange("k c o -> c k o"))
            cbsb = cpool.tile([128, 2, 1], F32, name="cbsb")
            nc.sync.dma_start(out=cbsb, in_=cbd.rearrange("(t p) one -> p t one", p=128))
            pbrep = cpool.tile([128, 256], F32, name="pbrep")
            nc.sync.dma_start(
                out=pbrep,
                in_=bass.AP(tensor=pbd.tensor, offset=0, ap=[[0, 128], [1, 256]]))
            epssb = cpool.tile([128, 1], F32, name="epssb")
            nc.vector.memset(epssb, EPS)
            gsb = cpool.tile([128, 16], F16, name="gsb")
            nc.sync.dma_start(out=gsb, in_=gd)
            masksb = cpool.tile([128, 16, 8], F16, name="masksb")
            nc.sync.dma_start(out=masksb, in_=mkd)
            sel16 = cpool.tile([128, 8], F16, name="sel16")
            nc.sync.dma_start(out=sel16, in_=seld)

            # routing-persistent tiles
            ubig = rpool.tile([128, 128, B], F16, name="ubig")
            vrep = rpool.tile([128, 16, 100], F16, name="vrep")
            b1sb = rpool.tile([128, 128, 100], F16, name="b1sb")
            v2sb = rpool.tile([8, 100, 16], F32, name="v2sb")

            # ---------- conv stages (scoped pools; freed before routing) ----
            with tc.tile_pool(name="work", bufs=2) as wpool, \
                 tc.tile_pool(name="acts", bufs=1) as apool:
                # stage A: conv1 [B,3,32,32] -> h [256, B, 24, 24]
                # xsb[(ci,kw), b, r, w] = x[b, ci, r, w+kw]; contraction over
                # (ci,kw)=27, accumulate over kh in PSUM.
                with tc.tile_pool(name="hpool", bufs=1) as hpool:
                    hsb = [hpool.tile([128, B, 24, 24], F16, name="hsb",
                                      tag=f"h{c}") for c in range(2)]
                    with tc.tile_pool(name="imp", bufs=1) as impool, \
                         tc.tile_pool(name="psc", bufs=2, space="PSUM") as pscpool:
                        xsf = impool.tile([27, B, 32, 24], F32, name="xsf")
                        for ci in range(3):
                            for kw in range(9):
                                src = bass.AP(
                                    tensor=xd.tensor,
                                    offset=ci * 1024 + kw,
                                    ap=[[3072, B], [32, 32], [1, 24]],
                                )
                                nc.sync.dma_start(
                                    out=xsf[ci * 9 + kw:ci * 9 + kw + 1], in_=src)
                        xsb = impool.tile([27, B, 32, 24], F16, name="xsb")
                        nc.vector.tensor_copy(xsb, xsf)

                        for oc in range(2):
                            for b in range(B):
                                for hh in range(2):
                                    ph = pscpool.tile([128, 288], F32, name="ph",
                                                      tag="pconv")
                                    for kh in range(9):
                                        nc.tensor.matmul(
                                            ph,
                                            lhsT=w1sb[:, kh, oc * 128:(oc + 1) * 128],
                                            rhs=xsb[:, b, kh + hh * 12:
                                                    kh + hh * 12 + 12, :].rearrange(
                                                        "c h w -> c (h w)"),
                                            start=(kh == 0), stop=(kh == 8),
                                        )
                                    nc.scalar.activation(
                                        hsb[oc][:, b, hh * 12:(hh + 1) * 12, :].rearrange(
                                            "p h w -> p (h w)"),
                                        ph, AF.Relu, bias=cbsb[:, oc],
                                    )

                    # stage B: conv2 (transposed) -> p2sb
                    p2sb = [apool.tile([128, 256], F32, name="p2sb",
                                       tag=f"p2sb{bp}") for bp in range(4)]
                    with tc.tile_pool(name="w2", bufs=2) as w2pool, \
                         tc.tile_pool(name="psc2", bufs=1, space="PSUM") as psc2pool:
                        p2ps = [psc2pool.tile([128, 256], F32, name="p2ps",
                                              tag=f"p2ps{bp}") for bp in range(4)]
                        nmm = [0, 0, 0, 0]
                        for g in range(9):
                            w2g = [w2pool.tile([128, 9, 256], F16, name="w2g",
                                               tag="w2g") for _ in range(2)]
                            for cic in range(2):
                                nc.sync.dma_start(out=w2g[cic],
                                                  in_=w2d[cic, :, g * 9:(g + 1) * 9, :])
                            for j in range(9):
                                khw = g * 9 + j
                                kh, kw = khw // 9, khw % 9
                                for cic in range(2):
                                    hshift = wpool.tile([128, B, 8, 8], F16,
                                                        name="hshift", tag="hshift")
                                    if cic == 0:
                                        nc.vector.tensor_copy(
                                            hshift,
                                            hsb[cic][:, :, kh:kh + 16:2, kw:kw + 16:2])
                                    else:
                                        nc.scalar.copy(
                                            hshift,
                                            hsb[cic][:, :, kh:kh + 16:2, kw:kw + 16:2])
                                    hflat = hshift.rearrange("p b h w -> p (b h w)")
                                    for bp in range(4):
                                        nc.tensor.matmul(
                                            p2ps[bp],
                                            lhsT=hflat[:, bp * 128:(bp + 1) * 128],
                                            rhs=w2g[cic][:, j, :],
                                            start=(nmm[bp] == 0), stop=(nmm[bp] == 161),
                                        )
                                        nmm[bp] += 1
                        for bp in range(4):
                            nc.vector.tensor_tensor(out=p2sb[bp], in0=p2ps[bp],
                                                    in1=pbrep, op=ALU.add)

                # stage C: squash over ow -> ub (bf16)
                ub = [apool.tile([128, 256], F16, name="ub", tag=f"ub{bp}")
                      for bp in range(4)]
                with tc.tile_pool(name="psn", bufs=2, space="PSUM") as psnpool:
                    for bp in range(4):
                        sq = wpool.tile([128, 256], F16, name="sq", tag="sq")
                        nc.vector.tensor_mul(sq, p2sb[bp], p2sb[bp])
                        n2ps = psnpool.tile([16, 256], F32, name="n2ps", tag="n2ps")
                        nc.tensor.matmul(n2ps, lhsT=gsb,
                                         rhs=sq, start=True, stop=True)
                        n2 = wpool.tile([16, 256], F32, name="n2", tag="n2")
                        nc.scalar.activation(n2, n2ps, AF.Copy)
                        r1 = wpool.tile([16, 256], F32, name="r1", tag="r1")
                        nc.vector.tensor_scalar_add(r1, in0=n2, scalar1=1.0)
                        nc.vector.reciprocal(r1, r1)
                        q = wpool.tile([16, 256], F32, name="q", tag="q")
                        nc.scalar.activation(q, n2, AF.Sqrt, bias=epssb[:16])
                        nc.vector.reciprocal(q, q)
                        f = wpool.tile([16, 256], F32, name="f", tag="f")
                        nc.vector.tensor_mul(f, n2, r1)
                        nc.vector.tensor_mul(f, f, q)
                        nc.sync.dma_start(out=fdram[bp], in_=f)
                        frep = wpool.tile([128, 256], F32, name="frep", tag="frep")
                        for grp in range(16):
                            nc.sync.dma_start(
                                out=frep[grp * 8:(grp + 1) * 8, :],
                                in_=bass.AP(tensor=fdram.tensor,
                                            offset=(bp * 16 + grp) * 256,
                                            ap=[[0, 8], [1, 256]]))
                        nc.vector.tensor_tensor(out=ub[bp], in0=p2sb[bp], in1=frep,
                                                op=ALU.mult)

                # stage D: u gathers -> ubig [128, cb, b]
                ubd = [wpool.tile([128, B, 64], F16, name="ubd", tag=f"ubd{H}")
                       for H in range(2)]
                for H in range(2):
                    for cp in range(2):
                        for b in range(B):
                            bp, bl = b // 2, b % 2
                            nc.sync.dma_start(
                                out=ubd[H][cp * 64:(cp + 1) * 64, b, :],
                                in_=ub[bp][bl * 64:(bl + 1) * 64,
                                           128 * H + 64 * cp:128 * H + 64 * cp + 64],
                            )
                for H in range(2):
                    nc.vector.tensor_copy(
                        ubig[:, 64 * H:64 * (H + 1), :],
                        bass.AP(tensor=ubd[H].tensor, offset=ubd[H].offset,
                                ap=[list(ubd[H].ap[0]), [1, 64], [64, B]]))

            # ---------- routing pools ----------
            vpool = stack.enter_context(tc.tile_pool(name="vsmall", bufs=1))
            wrpool = stack.enter_context(tc.tile_pool(name="wrp", bufs=10))
            uhppool = stack.enter_context(tc.tile_pool(name="uhp", bufs=2, space="PSUM"))
            psspool = stack.enter_context(tc.tile_pool(name="pss", bufs=1, space="PSUM"))
            ugpool = stack.enter_context(tc.tile_pool(name="ug", bufs=3))
            dmpool = stack.enter_context(tc.tile_pool(name="dmp", bufs=1))
            xpool = stack.enter_context(tc.tile_pool(name="xp", bufs=2))

            QS = (0, 512, 1024, 1536, 1600)

            def squash_dmaj(S, scale, final=False):
                """v = squash(S*scale); S psum [8, 1600] in d-major (d,o)."""
                S3 = S.rearrange("p (d o) -> p d o", d=16)
                sq = vpool.tile([8, 16, 100], F32, name="vsq", tag="vsq")
                nc.scalar.activation(sq.rearrange("p d o -> p (d o)"),
                                     S, AF.Square)
                nc.vector.tensor_tensor(out=sq[:, 0:8], in0=sq[:, 0:8],
                                        in1=sq[:, 8:16], op=ALU.add)
                nc.vector.tensor_tensor(out=sq[:, 0:4], in0=sq[:, 0:4],
                                        in1=sq[:, 4:8], op=ALU.add)
                nc.vector.tensor_tensor(out=sq[:, 0:2], in0=sq[:, 0:2],
                                        in1=sq[:, 2:4], op=ALU.add)
                n2 = vpool.tile([8, 100], F32, name="vn2", tag="vn2")
                nc.vector.tensor_tensor(out=n2, in0=sq[:, 0], in1=sq[:, 1],
                                        op=ALU.add)
                if scale != 1.0:
                    nc.vector.tensor_scalar_mul(n2, in0=n2, scalar1=scale * scale)
                r1 = vpool.tile([8, 100], F32, name="vr1", tag="vr1")
                nc.vector.tensor_scalar_add(r1, in0=n2, scalar1=1.0)
                nc.vector.reciprocal(r1, r1)
                q = vpool.tile([8, 100], F32, name="vq", tag="vq")
                nc.scalar.activation(q, n2, AF.Sqrt, bias=epssb[:8])
                nc.vector.reciprocal(q, q)
                f = vpool.tile([8, 100], F32, name="vf", tag="vf")
                nc.vector.tensor_mul(f, n2, r1)
                nc.vector.tensor_mul(f, f, q)
                if scale != 1.0:
                    nc.vector.tensor_scalar_mul(f, in0=f, scalar1=scale)
                if final:
                    nc.vector.tensor_tensor(
                        out=v2sb, in0=S3.transpose([0, 2, 1]),
                        in1=f.unsqueeze(2).broadcast_to([8, 100, 16]),
                        op=ALU.mult)
                    nc.sync.dma_start(out=vout, in_=v2sb)
                else:
                    v16 = vpool.tile([8, 16, 100], F16, name="v16", tag="v16")
                    nc.vector.tensor_tensor(
                        out=v16, in0=S3,
                        in1=f.unsqueeze(1).broadcast_to([8, 16, 100]),
                        op=ALU.mult)
                    nc.sync.dma_start(out=vdram,
                                      in_=v16.rearrange("p d o -> p (d o)"))
                    nc.sync.dma_start(
                        out=vrep.rearrange("p d o -> p (d o)"),
                        in_=bass.AP(tensor=vdram.tensor, offset=0,
                                    ap=[[0, 16], [1600, 8], [1, 1600]]))

            # ---------- pass 0: s0 = sum_i u_hat ----------
            s0ps = psspool.tile([8, 1600], F32, name="sps", tag="sps")
            for cb in range(128):
                wrt = wrpool.tile([128, 1600], F16, name="wrt", tag="wrt")
                eng = nc.sync if cb % 2 == 0 else nc.scalar
                eng.dma_start(out=wrt, in_=wrd[cb])
                for q in range(4):
                    n0, n1 = QS[q], QS[q + 1]
                    nc.tensor.matmul(s0ps[:, n0:n1],
                                     lhsT=ubig[:, cb, :],
                                     rhs=wrt[:, n0:n1],
                                     start=(cb == 0), stop=(cb == 127))
            squash_dmaj(s0ps, 0.01)

            # ---------- passes 1, 2 (software-pipelined) ----------
            NG = 128 // G

            GGD = slice(G // 2, G - 1)   # chunks 4..6 (DVE)
            GG7 = slice(G - 1, G)        # chunk 7 (GpSimd, contiguous)
            GD7 = slice(0, G - 1)        # chunks 0..6

            def head_half(uhg, dm, hh, gvb):
                """dm = uh*v and fold tree (in place), split DVE / GpSimd by
                whole chunks so every op reads contiguous rows."""
                if hh == 0:
                    gs = slice(0, G // 2)
                    nc.vector.tensor_tensor(out=dm[:, gs], in0=uhg[:, gs],
                                            in1=gvb[:, gs], op=ALU.mult)
                    nc.vector.tensor_tensor(out=dm[:, gs, 0:8], in0=dm[:, gs, 0:8],
                                            in1=dm[:, gs, 8:16], op=ALU.add)
                else:
                    nc.vector.tensor_tensor(out=dm[:, GGD], in0=uhg[:, GGD],
                                            in1=gvb[:, GGD], op=ALU.mult)
                    nc.gpsimd.tensor_tensor(out=dm[:, GG7], in0=uhg[:, GG7],
                                            in1=gvb[:, GG7], op=ALU.mult)
                    nc.vector.tensor_tensor(out=dm[:, GGD, 0:8],
                                            in0=dm[:, GGD, 0:8],
                                            in1=dm[:, GGD, 8:16], op=ALU.add)
                    nc.gpsimd.tensor_tensor(out=dm[:, GG7, 0:8],
                                            in0=dm[:, GG7, 0:8],
                                            in1=dm[:, GG7, 8:16], op=ALU.add)
                    nc.vector.tensor_tensor(out=dm[:, GD7, 0:4],
                                            in0=dm[:, GD7, 0:4],
                                            in1=dm[:, GD7, 4:8], op=ALU.add)
                    nc.gpsimd.tensor_tensor(out=dm[:, GG7, 0:4],
                                            in0=dm[:, GG7, 0:4],
                                            in1=dm[:, GG7, 4:8], op=ALU.add)
                    nc.gpsimd.tensor_tensor(out=dm[:, :, 0:2], in0=dm[:, :, 0:2],
                                            in1=dm[:, :, 2:4], op=ALU.add)

            for t in (1, 2):
                sps = psspool.tile([8, 1600], F32, name="sps", tag="sps")
                st = {}
                for gi in range(NG + 2):
                    # ---- softmax tail for group gi-1 ----
                    if 1 <= gi <= NG:
                        s = st[gi - 1]
                        e = xpool.tile([128, G, 100], F16, name="e", tag="e")
                        nc.scalar.activation(e.rearrange("p g o -> p (g o)"),
                                             s["logit"].rearrange("p g o -> p (g o)"),
                                             AF.Exp)
                        zf = xpool.tile([128, G], F32, name="zf", tag="zf")
                        nc.vector.tensor_reduce(zf, e, axis=AX.X, op=ALU.add)
                        zi = xpool.tile([128, G], F32, name="zi", tag="zi")
                        nc.vector.reciprocal(zi, zf)
                        zsel = xpool.tile([128, G, 8], F16, name="zsel", tag="zsel")
                        nc.vector.tensor_tensor(
                            out=zsel,
                            in0=sel16.unsqueeze(1).broadcast_to([128, G, 8]),
                            in1=zi.unsqueeze(2).broadcast_to([128, G, 8]),
                            op=ALU.mult)
                        s["zsel"] = zsel
                        uhg = s["uhg"]
                        eb = e.unsqueeze(2).broadcast_to([128, G, 16, 100])
                        nc.vector.tensor_tensor(out=uhg[:, GD7], in0=uhg[:, GD7],
                                                in1=eb[:, GD7], op=ALU.mult)
                        nc.gpsimd.tensor_tensor(out=uhg[:, GG7], in0=uhg[:, GG7],
                                                in1=eb[:, GG7], op=ALU.mult)
                    # ---- s-matmuls for group gi-2 ----
                    if 2 <= gi:
                        sp = st[gi - 2]
                        pf = sp["uhg"].rearrange("p g d o -> p (g d o)")
                        for j in range(G):
                            cb = sp["g0"] + j
                            for q in range(4):
                                n0, n1 = QS[q], QS[q + 1]
                                nc.tensor.matmul(
                                    sps[:, n0:n1], lhsT=sp["zsel"][:, j, :],
                                    rhs=pf[:, j * 1600 + n0:j * 1600 + n1],
                                    start=(cb == 0), stop=(cb == 127))
                        del st[gi - 2]
                    # ---- produce group gi + logit head ----
                    if gi < NG:
                        g0 = gi * G
                        # block-diag operand for this group
                        bdg = xpool.tile([128, G, 16, 8], F16, name="bdg", tag="bdg")
                        nc.vector.tensor_tensor(
                            out=bdg,
                            in0=ubig[:, g0:g0 + G].unsqueeze(2).broadcast_to(
                                [128, G, 16, 8]),
                            in1=masksb.unsqueeze(1).broadcast_to([128, G, 16, 8]),
                            op=ALU.mult)
                        uhg = ugpool.tile([128, G, 16, 100], F16, name="uhg",
                                          tag="uhg")
                        uhg_f = uhg.rearrange("p g d o -> p (g d o)")
                        dm = dmpool.tile([128, G, 16, 100], F16, name="dm", tag="dm")
                        gvb = vrep.unsqueeze(1).broadcast_to([128, G, 16, 100])
                        for j in range(G):
                            cb = g0 + j
                            wrt = wrpool.tile([128, 1600], F16, name="wrt", tag="wrt")
                            nc.sync.dma_start(out=wrt, in_=wrd[cb])
                            lhsT = bdg[:, j].rearrange("p i b -> p (i b)")
                            for h in range(2):
                                ps = uhppool.tile([128, 800], F32, name="uhps",
                                                  tag="uhps")
                                nc.tensor.matmul(ps[:, 0:512], lhsT=lhsT,
                                                 rhs=wrt[:, h * 800:h * 800 + 512],
                                                 start=True, stop=True)
                                nc.tensor.matmul(ps[:, 512:800], lhsT=lhsT,
                                                 rhs=wrt[:, h * 800 + 512:(h + 1) * 800],
                                                 start=True, stop=True)
                                nc.scalar.activation(
                                    uhg_f[:, j * 1600 + h * 800:j * 1600 + (h + 1) * 800],
                                    ps, AF.Copy)
                            if j == G // 2 - 1:
                                head_half(uhg, dm, 0, gvb)
                        head_half(uhg, dm, 1, gvb)
                        if t == 1:
                            logit = b1sb[:, g0:g0 + G]
                            nc.gpsimd.tensor_tensor(out=logit, in0=dm[:, :, 0],
                                                    in1=dm[:, :, 1], op=ALU.add)
                        else:
                            nc.gpsimd.tensor_tensor(out=dm[:, :, 0], in0=dm[:, :, 0],
                                                    in1=dm[:, :, 1], op=ALU.add)
                            logit = xpool.tile([128, G, 100], F16, name="lgt",
                                               tag="lgt")
                            nc.vector.tensor_tensor(out=logit, in0=dm[:, :, 0],
                                                    in1=b1sb[:, g0:g0 + G],
                                                    op=ALU.add)
                        st[gi] = {"g0": g0, "uhg": uhg, "logit": logit}
                squash_dmaj(sps, 1.0, final=(t == 2))

    nc.compile()
    return nc


def _host_prep(x, conv_w, conv_b, pcap_w, pcap_b, W):
    x = np.ascontiguousarray(np.asarray(x, np.float32))
    conv_w = np.asarray(conv_w, np.float32)
    conv_b = np.asarray(conv_b, np.float32)
    pcap_w = np.asarray(pcap_w, np.float32)
    pcap_b = np.asarray(pcap_b, np.float32)
    W = np.asarray(W, np.float32)

    # w1t[kh, (ci,kw), oc] = conv_w[oc, ci, kh, kw]
    w1t = np.ascontiguousarray(
        conv_w.transpose(2, 1, 3, 0).reshape(9, 27, 256)
    ).astype(np.float16)
    cb = np.ascontiguousarray(conv_b.reshape(256, 1))
    w2t = np.ascontiguousarray(
        pcap_w.transpose(1, 2, 3, 0).reshape(2, 128, 81, 256)
    ).astype(np.float16)
    pb = np.ascontiguousarray(pcap_b.reshape(1, 256))
    # wr[cb=(H,r)][p=(cp,oh,ow)][(d,o)] = W[o, (128H+64cp+r)*8+oh, d, ow]
    arr = W.transpose(1, 3, 0, 2)                # [i=2048, k=8, o=100, d=16]
    arr = arr.reshape(2, 2, 64, 8, 8, 100, 16)   # [H, cp, r, oh, k, o, d]
    arr = arr.transpose(0, 2, 1, 3, 4, 6, 5)     # [H, r, cp, oh, k, d, o]
    wr = np.ascontiguousarray(arr.reshape(128, 128, 1600)).astype(np.float16)

    mask = np.zeros((128, 16, 8), np.float32)
    for p in range(128):
        mask[p, p // 8, :] = 1.0
    mask = mask.astype(np.float16)
    sel = np.zeros((128, 8), np.float32)
    for p in range(128):
        sel[p, p % 8] = 1.0
    sel = sel.astype(np.float16)
    g = np.zeros((128, 16), np.float32)
    for p in range(128):
        g[p, p // 8] = 1.0
    g = g.astype(np.float16)

    shared = {"w1t": w1t, "cb": cb, "w2t": w2t, "pb": pb, "wr": wr,
              "mask": mask, "sel": sel, "gmat": g}
    in_maps = []
    for c in range(N_CORES):
        m = dict(shared)
        m["x_sh"] = np.ascontiguousarray(x[c * B:(c + 1) * B])
        in_maps.append(m)
    return in_maps


def run(inputs, trace=False, **kw):
    key = "nc"
    if key not in _CACHE:
        _CACHE[key] = _build()
    nc = _CACHE[key]
    in_maps = _host_prep(**inputs)
    res = bass_utils.run_bass_kernel_spmd(
        nc, in_maps, core_ids=list(range(N_CORES)), trace=trace, **kw)
    return res


def kernel(**inputs):
    res = run(inputs)
    v = np.concatenate([res.results[i]["v_out"] for i in range(N_CORES)], axis=0)
    return v



# revision 21
# speedup vs baseline: 1.1673x; 1.1673x over previous
"""CapsNet-CIFAR100 forward on 8 Trainium2 NeuronCores.

Hybrid sharding: conv stem data-parallel (8 images/core), dynamic routing
out-capsule-sharded (each core: all 64 images x 13 o-slots, W shard
SBUF/stream-resident).  u_hat (26M elems) is never materialized:

  pass 0:  s0 = 0.01 * sum_ik W u           (PE, (i,k)-contraction)
  dm path: y = sum_d W v  (PE, half of u_hat), dm = sum_k y*u (DVE folds)
  softmax over o is GLOBAL: z partial-summed per core, AllReduce'd.
  s path:  cu = c*u (DVE), s = sum_ik W cu  (PE, (i,k)-contraction)

Index maps (per core):  i = co*8+oh, k = ow;  b in [64];  o in [13] slots.
  P1 = oh*16 + b%16 (quads q=b//16)   "dm layout"  [p, (o, q, co)]
  P2 = co%128 (halves h=co//128)      "s layout"   [p, (h, oh, ow, ...)]
Collectives: AllGather of u (262KB/core) after conv; AllReduce of
z=[128,1024] f16 per routing pass.
"""

from contextlib import ExitStack

import numpy as np
import concourse.bass as bass
import concourse.mybir as mybir
import concourse.tile as tile
from concourse import bacc
from concourse import bass_utils

F32 = mybir.dt.float32
F16 = mybir.dt.float16
AF = mybir.ActivationFunctionType
ALU = mybir.AluOpType
AX = mybir.AxisListType

N_CORES = 8
B = 8            # conv batch per core
NO = 13          # o-slots per core (padded)
SIZES = [13, 13, 13, 13, 12, 12, 12, 12]
OFFS = [0, 13, 26, 39, 52, 64, 76, 88]
EPS = 1e-8

_CACHE = {}


def _build():
    nc = bacc.Bacc("TRN2", target_bir_lowering=False, debug=False,
                   num_devices=N_CORES)
    RG = [list(range(N_CORES))]

    # conv inputs (as baseline)
    xd = nc.dram_tensor("x_sh", [B, 3, 32, 32], F32, kind="ExternalInput").ap()
    w1d = nc.dram_tensor("w1t", [9, 27, 256], F16, kind="ExternalInput").ap()
    cbd = nc.dram_tensor("cb", [256, 1], F32, kind="ExternalInput").ap()
    w2d = nc.dram_tensor("w2t", [2, 128, 81, 256], F16, kind="ExternalInput").ap()
    pbd = nc.dram_tensor("pb", [1, 256], F32, kind="ExternalInput").ap()
    # routing inputs
    wyd = nc.dram_tensor("wy", [NO, 128, 2048], F16, kind="ExternalInput").ap()
    wsd = nc.dram_tensor("ws", [NO, 128, 2048], F16, kind="ExternalInput").ap()
    ws0d = nc.dram_tensor("ws0", [8, 128, 16 * NO * 16], F16, kind="ExternalInput").ap()
    rsd = nc.dram_tensor("repsel", [16, 128], F16, kind="ExternalInput").ap()
    obd = nc.dram_tensor("obias", [128, NO], F16, kind="ExternalInput").ap()
    bmd = nc.dram_tensor("blkmask", [128, 128], F16, kind="ExternalInput").ap()
    idd = nc.dram_tensor("ident", [128, 128], F16, kind="ExternalInput").ap()
    # scratch / comm
    fdram = nc.dram_tensor("fscratch", [4, 16, 256], F32, kind="Internal").ap()
    ubd_t = nc.dram_tensor("ub_d", [B * 16384], F16, kind="Internal").ap()
    uall_t = nc.dram_tensor("uall_d", [64 * 16384], F16, kind="Internal",
                            addr_space="Shared").ap()
    zbd = nc.dram_tensor("zb_d", [128 * 1024], F16, kind="Internal").ap()
    zrd = nc.dram_tensor("zr_d", [128 * 1024], F16, kind="Internal",
                         addr_space="Shared").ap()
    vout = nc.dram_tensor("v_out", [64, NO, 16], F32, kind="ExternalOutput").ap()

    with tile.TileContext(nc) as tc:
        with ExitStack() as stack:
            cpool = stack.enter_context(tc.tile_pool(name="consts", bufs=1))

            # ---------- shared constants ----------
            w1sb = cpool.tile([27, 9, 256], F16, name="w1sb")
            nc.sync.dma_start(out=w1sb, in_=w1d.rearrange("k c o -> c k o"))
            cbsb = cpool.tile([128, 2, 1], F32, name="cbsb")
            nc.sync.dma_start(out=cbsb, in_=cbd.rearrange("(t p) one -> p t one", p=128))
            pbrep = cpool.tile([128, 256], F32, name="pbrep")
            nc.sync.dma_start(
                out=pbrep,
                in_=bass.AP(tensor=pbd.tensor, offset=0, ap=[[0, 128], [1, 256]]))
            epssb = cpool.tile([128, 1], F32, name="epssb")
            nc.vector.memset(epssb, EPS)
            gsb = cpool.tile([128, 16], F16, name="gsb")
            # g[p, p//8] = 1 selector for squash-u (built on host side of blkmask? no: iota)
            # build via memset+iota is awkward; reuse host input blkmask? separate input:
            # we fold it into obias? -> simplest: device-side from ident via DMA is messy.
            # Use dedicated host input below (gmat).
            bmsb = cpool.tile([128, 128], F16, name="bmsb")
            nc.sync.dma_start(out=bmsb, in_=bmd)
            idsb = cpool.tile([128, 128], F16, name="idsb")
            nc.sync.dma_start(out=idsb, in_=idd)
            obsb = cpool.tile([128, NO], F16, name="obsb")
            nc.sync.dma_start(out=obsb, in_=obd)
            rssb = cpool.tile([16, 128], F16, name="rssb")
            nc.sync.dma_start(out=rssb, in_=rsd)

            gmd = nc.dram_tensor("gmat", [128, 16], F16, kind="ExternalInput").ap()
            nc.sync.dma_start(out=gsb, in_=gmd)

            # ---------- conv stages (baseline, stages A-C) ----------
            with tc.tile_pool(name="work", bufs=2) as wpool, \
                 tc.tile_pool(name="acts", bufs=1) as apool:
                with tc.tile_pool(name="hpool", bufs=1) as hpool:
                    hsb = [hpool.tile([128, B, 24, 24], F16, name="hsb",
                                      tag=f"h{c}") for c in range(2)]
                    with tc.tile_pool(name="imp", bufs=1) as impool, \
                         tc.tile_pool(name="psc", bufs=2, space="PSUM") as pscpool:
                        xsf = impool.tile([27, B, 32, 24], F32, name="xsf")
                        for ci in range(3):
                            for kw in range(9):
                                src = bass.AP(
                                    tensor=xd.tensor,
                                    offset=ci * 1024 + kw,
                                    ap=[[3072, B], [32, 32], [1, 24]],
                                )
                                nc.sync.dma_start(
                                    out=xsf[ci * 9 + kw:ci * 9 + kw + 1], in_=src)
                        xsb = impool.tile([27, B, 32, 24], F16, name="xsb")
                        nc.vector.tensor_copy(xsb, xsf)

                        for oc in range(2):
                            for b in range(B):
                                for hh in range(2):
                                    ph = pscpool.tile([128, 288], F32, name="ph",
                                                      tag="pconv")
                                    for kh in range(9):
                                        nc.tensor.matmul(
                                            ph,
                                            lhsT=w1sb[:, kh, oc * 128:(oc + 1) * 128],
                                            rhs=xsb[:, b, kh + hh * 12:
                                                    kh + hh * 12 + 12, :].rearrange(
                                                        "c h w -> c (h w)"),
                                            start=(kh == 0), stop=(kh == 8),
                                        )
                                    nc.scalar.activation(
                                        hsb[oc][:, b, hh * 12:(hh + 1) * 12, :].rearrange(
                                            "p h w -> p (h w)"),
                                        ph, AF.Relu, bias=cbsb[:, oc],
                                    )

                    # stage B: conv2 (transposed) -> p2sb
                    p2sb = [apool.tile([128, 256], F32, name="p2sb",
                                       tag=f"p2sb{bp}") for bp in range(4)]
                    with tc.tile_pool(name="w2", bufs=2) as w2pool, \
                         tc.tile_pool(name="psc2", bufs=1, space="PSUM") as psc2pool:
                        p2ps = [psc2pool.tile([128, 256], F32, name="p2ps",
                                              tag=f"p2ps{bp}") for bp in range(4)]
                        nmm = [0, 0, 0, 0]
                        for g in range(9):
                            w2g = [w2pool.tile([128, 9, 256], F16, name="w2g",
                                               tag="w2g") for _ in range(2)]
                            for cic in range(2):
                                nc.sync.dma_start(out=w2g[cic],
                                                  in_=w2d[cic, :, g * 9:(g + 1) * 9, :])
                            for j in range(9):
                                khw = g * 9 + j
                                kh, kw = khw // 9, khw % 9
                                for cic in range(2):
                                    hshift = wpool.tile([128, B, 8, 8], F16,
                                                        name="hshift", tag="hshift")
                                    if cic == 0:
                                        nc.vector.tensor_copy(
                                            hshift,
                                            hsb[cic][:, :, kh:kh + 16:2, kw:kw + 16:2])
                                    else:
                                        nc.scalar.copy(
                                            hshift,
                                            hsb[cic][:, :, kh:kh + 16:2, kw:kw + 16:2])
                                    hflat = hshift.rearrange("p b h w -> p (b h w)")
                                    for bp in range(4):
                                        nc.tensor.matmul(
                                            p2ps[bp],
                                            lhsT=hflat[:, bp * 128:(bp + 1) * 128],
                                            rhs=w2g[cic][:, j, :],
                                            start=(nmm[bp] == 0), stop=(nmm[bp] == 161),
                                        )
                                        nmm[bp] += 1
                        for bp in range(4):
                            nc.vector.tensor_tensor(out=p2sb[bp], in0=p2ps[bp],
                                                    in1=pbrep, op=ALU.add)

                # stage C: squash over ow -> ub (bf16)
                ub = [apool.tile([128, 256], F16, name="ub", tag=f"ub{bp}")
                      for bp in range(4)]
                with tc.tile_pool(name="psn", bufs=2, space="PSUM") as psnpool:
                    for bp in range(4):
                        sq = wpool.tile([128, 256], F16, name="sq", tag="sq")
                        nc.vector.tensor_mul(sq, p2sb[bp], p2sb[bp])
                        n2ps = psnpool.tile([16, 256], F32, name="n2ps", tag="n2ps")
                        nc.tensor.matmul(n2ps, lhsT=gsb,
                                         rhs=sq, start=True, stop=True)
                        n2 = wpool.tile([16, 256], F32, name="n2", tag="n2")
                        nc.scalar.activation(n2, n2ps, AF.Copy)
                        r1 = wpool.tile([16, 256], F32, name="r1", tag="r1")
                        nc.vector.tensor_scalar_add(r1, in0=n2, scalar1=1.0)
                        nc.vector.reciprocal(r1, r1)
                        q = wpool.tile([16, 256], F32, name="q", tag="q")
                        nc.scalar.activation(q, n2, AF.Sqrt, bias=epssb[:16])
                        nc.vector.reciprocal(q, q)
                        f = wpool.tile([16, 256], F32, name="f", tag="f")
                        nc.vector.tensor_mul(f, n2, r1)
                        nc.vector.tensor_mul(f, f, q)
                        nc.sync.dma_start(out=fdram[bp], in_=f)
                        frep = wpool.tile([128, 256], F32, name="frep", tag="frep")
                        for grp in range(16):
                            nc.sync.dma_start(
                                out=frep[grp * 8:(grp + 1) * 8, :],
                                in_=bass.AP(tensor=fdram.tensor,
                                            offset=(bp * 16 + grp) * 256,
                                            ap=[[0, 8], [1, 256]]))
                        nc.vector.tensor_tensor(out=ub[bp], in0=p2sb[bp], in1=frep,
                                                op=ALU.mult)

                # export u to DRAM: ub_d[b*16384 + oh*2048 + ow*256 + co]
                # (co contiguous so every DMA run is 512B)
                for bp in range(4):
                    for bl in range(2):
                        dst = bass.AP(tensor=ubd_t.tensor,
                                      offset=(bp * 2 + bl) * 16384,
                                      ap=[[256, 64], [1, 256]])
                        nc.sync.dma_start(out=dst,
                                          in_=ub[bp][bl * 64:(bl + 1) * 64, :])

            # ---------- u AllGather ----------
            nc.gpsimd.collective_compute(
                "AllGather", ALU.bypass, replica_groups=RG,
                ins=[ubd_t.opt()], outs=[uall_t.opt()])

            # ---------- routing-persistent tiles + u relayouts ----------
            rpool = stack.enter_context(tc.tile_pool(name="rp", bufs=1))
            u_y = rpool.tile([128, 4, 256, 8], F16, name="u_y")
            u_s = rpool.tile([128, 2, 8, 8, 64], F16, name="u_s")
            with tc.tile_pool(name="rly", bufs=1) as rlpool, \
                 tc.tile_pool(name="rlps", bufs=4, space="PSUM") as rlps:
                ug = rlpool.tile([64, 16384], F16, name="ug")
                nc.sync.dma_start(
                    out=ug,
                    in_=bass.AP(tensor=uall_t.tensor, offset=0,
                                ap=[[16384, 64], [1, 16384]]))
                # T1: u_s[p=co%128, (h,oh,ow,b)] via per-(h,oh,ow) transposes
                ugv = ug.rearrange("b (g w c) -> b g w c", g=8, w=8)
                for h in range(2):
                    for oh in range(8):
                        for ow in range(8):
                            t1 = rlps.tile([128, 64], F16, name="t1", tag="t1")
                            nc.tensor.transpose(
                                t1, ugv[:, oh, ow, h * 128:(h + 1) * 128],
                                idsb[:64, :64])
                            nc.scalar.activation(u_s[:, h, oh, ow, :], t1, AF.Copy)
                # u_mid[co, (h, k, q, oh, b16)] = u_s[co, h, oh, k, q*16+b16]
                umid = rlpool.tile([128, 2, 8, 4, 8, 16], F16, name="umid")
                nc.vector.tensor_copy(
                    umid,
                    u_s.rearrange("p h oh k (q s) -> p h k q oh s", q=4))
                # T2: u_y[p=(oh,b16), (q, co, k)]
                for h in range(2):
                    for k in range(8):
                        for q in range(4):
                            t2 = rlps.tile([128, 128], F16, name="t2", tag="t2")
                            nc.tensor.transpose(t2, umid[:, h, k, q].rearrange(
                                "p a s -> p (a s)"), idsb)
                            nc.scalar.activation(
                                u_y[:, q, h * 128:(h + 1) * 128, k],
                                t2, AF.Copy)

            dm0 = rpool.tile([128, NO, 4, 256], F16, name="dm0")
            dm1 = rpool.tile([128, NO, 4, 256], F16, name="dm1")
            vrep = rpool.tile([128, NO, 64], F16, name="vrep")
            s_all = rpool.tile([64, NO, 16], F32, name="s_all")
            zsum = rpool.tile([128, 1024], F16, name="zsum")
            zf32 = rpool.tile([128, 1024], F32, name="zf32")
            zi16 = rpool.tile([128, 4, 256], F16, name="zi16")
            zp = rpool.tile([128, 4, 256], F16, name="zp")

            vpool = stack.enter_context(tc.tile_pool(name="vp", bufs=1))
            vpsp = stack.enter_context(tc.tile_pool(name="vpsp", bufs=1, space="PSUM"))
            vpsp2 = stack.enter_context(tc.tile_pool(name="vpsp2", bufs=1, space="PSUM"))

            def squash(t):
                """s_all [64, NO, 16] f32 -> v; t=2 writes vout, else vrep."""
                sq = vpool.tile([64, NO, 16], F32, name="ssq", tag="ssq")
                nc.vector.tensor_mul(sq, s_all, s_all)
                n2 = vpool.tile([64, NO], F32, name="sn2", tag="sn2")
                nc.vector.tensor_reduce(n2, sq, axis=AX.X, op=ALU.add)
                r1 = vpool.tile([64, NO], F32, name="sr1", tag="sr1")
                nc.vector.tensor_scalar_add(r1, in0=n2, scalar1=1.0)
                nc.vector.reciprocal(r1, r1)
                qq = vpool.tile([64, NO], F32, name="sqq", tag="sqq")
                nc.scalar.activation(qq, n2, AF.Sqrt, bias=epssb[:64])
                nc.vector.reciprocal(qq, qq)
                ff = vpool.tile([64, NO], F32, name="sff", tag="sff")
                nc.vector.tensor_mul(ff, n2, r1)
                nc.vector.tensor_mul(ff, ff, qq)
                vv = vpool.tile([64, NO, 16], F32, name="svv", tag="svv")
                nc.vector.tensor_tensor(
                    out=vv, in0=s_all,
                    in1=ff.unsqueeze(2).broadcast_to([64, NO, 16]), op=ALU.mult)
                if t == 2:
                    nc.sync.dma_start(out=vout, in_=vv)
                else:
                    # vrep[p=(oh,d), (o,b)] via transpose + replication matmul
                    vb16 = vpool.tile([64, NO, 16], F16, name="svb", tag="svb")
                    nc.vector.tensor_copy(vb16, vv)
                    for o in range(NO):
                        tp = vpsp.tile([16, 64], F16, name="vtp", tag="vtp")
                        nc.tensor.transpose(tp, vb16[:, o, :], idsb[:64, :64])
                        vts = vpool.tile([16, 64], F16, name="vts", tag="vts")
                        nc.scalar.activation(vts, tp, AF.Copy)
                        rp = vpsp2.tile([128, 64], F32, name="vrp", tag="vrp")
                        nc.tensor.matmul(rp, lhsT=rssb, rhs=vts,
                                         start=True, stop=True)
                        nc.scalar.activation(vrep[:, o, :], rp, AF.Copy)

            # ---------- pass 0 ----------
            with tc.tile_pool(name="p0", bufs=2) as p0pool, \
                 tc.tile_pool(name="p0c", bufs=1) as p0cpool, \
                 tc.tile_pool(name="p0ps", bufs=1, space="PSUM") as p0psp:
                cu0 = p0cpool.tile([128, 2, 8, 8, 64], F16, name="cu0")
                nc.vector.tensor_scalar_mul(cu0, in0=u_s, scalar1=0.01)
                s0ps = p0psp.tile([64, NO * 16], F32, name="s0ps")
                for g in range(8):
                    w0t = p0pool.tile([128, 16, NO * 16], F16, name="w0t", tag="w0t")
                    nc.sync.dma_start(
                        out=w0t,
                        in_=bass.AP(tensor=ws0d.tensor, offset=g * 128 * 16 * NO * 16,
                                    ap=[[16 * NO * 16, 128], [1, 16 * NO * 16]]))
                    for j in range(16):
                        ch = g * 16 + j
                        h, oh, ow = ch // 64, (ch // 8) % 8, ch % 8
                        nc.tensor.matmul(
                            s0ps, lhsT=cu0[:, h, oh, ow, :], rhs=w0t[:, j, :],
                            start=(ch == 0), stop=(ch == 127))
                nc.scalar.activation(s_all.rearrange("b o d -> b (o d)"), s0ps,
                                     AF.Copy)
                squash(0)

            # ---------- passes 1, 2 ----------
            wypool = stack.enter_context(tc.tile_pool(name="wyp", bufs=2))
            wspool = stack.enter_context(tc.tile_pool(name="wsp", bufs=2))
            ypool = stack.enter_context(tc.tile_pool(name="yp", bufs=2))
            cupool = stack.enter_context(tc.tile_pool(name="cup", bufs=2))
            ctpool = stack.enter_context(tc.tile_pool(name="ctp", bufs=2))
            vbpool = stack.enter_context(tc.tile_pool(name="vbp", bufs=2))
            ypsp = stack.enter_context(tc.tile_pool(name="ypsp", bufs=2, space="PSUM"))
            tpsp = stack.enter_context(tc.tile_pool(name="tpsp", bufs=2, space="PSUM"))
            spsp = stack.enter_context(tc.tile_pool(name="spsp", bufs=2, space="PSUM"))

            for t in (1, 2):
                dmt = dm0 if t == 1 else dm1
                # ---- y / dm stage ----
                for o in range(NO):
                    wyt = wypool.tile([128, 2048], F16, name="wyt", tag="wyt")
                    nc.sync.dma_start(
                        out=wyt,
                        in_=bass.AP(tensor=wyd.tensor, offset=o * 128 * 2048,
                                    ap=[[2048, 128], [1, 2048]]))
                    vblk = vbpool.tile([128, 4, 8, 16], F16, name="vblk", tag="vblk")
                    nc.vector.tensor_tensor(
                        out=vblk,
                        in0=vrep[:, o, :].rearrange("p (q s) -> p q s", q=4)
                            .unsqueeze(2).broadcast_to([128, 4, 8, 16]),
                        in1=bmsb.rearrange("p (a s) -> p a s", a=8)
                            .unsqueeze(1).broadcast_to([128, 4, 8, 16]),
                        op=ALU.mult)
                    y16 = ypool.tile([128, 4, 256, 8], F16, name="y16", tag="y16")
                    for q in range(4):
                        for cc in range(4):
                            yps = ypsp.tile([128, 512], F32, name="yps", tag="yps")
                            nc.tensor.matmul(
                                yps,
                                lhsT=vblk[:, q].rearrange("p a s -> p (a s)"),
                                rhs=wyt[:, cc * 512:(cc + 1) * 512],
                                start=True, stop=True)
                            nc.scalar.activation(
                                y16[:, q, cc * 64:(cc + 1) * 64, :].rearrange(
                                    "p c k -> p (c k)"),
                                yps, AF.Copy)
                    nc.vector.tensor_tensor(out=y16, in0=y16, in1=u_y, op=ALU.mult)
                    eng = nc.vector if o % 2 == 0 else nc.gpsimd
                    eng.tensor_tensor(out=y16[:, :, :, 0:4], in0=y16[:, :, :, 0:4],
                                      in1=y16[:, :, :, 4:8], op=ALU.add)
                    eng.tensor_tensor(out=y16[:, :, :, 0:2], in0=y16[:, :, :, 0:2],
                                      in1=y16[:, :, :, 2:4], op=ALU.add)
                    nc.vector.tensor_tensor(out=dmt[:, o], in0=y16[:, :, :, 0],
                                            in1=y16[:, :, :, 1], op=ALU.add)

                if t == 1:
                    nc.gpsimd.tensor_tensor(
                        out=dm0, in0=dm0,
                        in1=obsb.unsqueeze(2).unsqueeze(3)
                            .broadcast_to([128, NO, 4, 256]),
                        op=ALU.add)
                else:
                    nc.vector.tensor_tensor(out=dm0, in0=dm0, in1=dm1, op=ALU.add)

                # e = exp(logits) into dm1 (overwrites)
                nc.scalar.activation(dm1.rearrange("p o q c -> p (o q c)"),
                                     dm0.rearrange("p o q c -> p (o q c)"), AF.Exp)
                # z partial = sum_o e  (gpsimd chain)
                nc.gpsimd.tensor_tensor(out=zp, in0=dm1[:, 0], in1=dm1[:, 1],
                                        op=ALU.add)
                for o in range(2, NO):
                    nc.gpsimd.tensor_tensor(out=zp, in0=zp, in1=dm1[:, o],
                                            op=ALU.add)
                nc.gpsimd.dma_start(
                    out=bass.AP(tensor=zbd.tensor, offset=0,
                                ap=[[1024, 128], [1, 1024]]),
                    in_=zp.rearrange("p q c -> p (q c)"))
                nc.gpsimd.collective_compute(
                    "AllReduce", ALU.add, replica_groups=RG,
                    ins=[zbd.opt()], outs=[zrd.opt()])
                nc.gpsimd.dma_start(
                    out=zsum,
                    in_=bass.AP(tensor=zrd.tensor, offset=0,
                                ap=[[1024, 128], [1, 1024]]))
                nc.vector.tensor_copy(zf32, zsum)
                nc.vector.reciprocal(zf32, zf32)
                nc.vector.tensor_copy(zi16.rearrange("p q c -> p (q c)"), zf32)
                # c = e * zi  (in place on dm1)
                nc.vector.tensor_tensor(
                    out=dm1, in0=dm1,
                    in1=zi16.unsqueeze(1).broadcast_to([128, NO, 4, 256]),
                    op=ALU.mult)

                # ---- s stage ----
                for o in range(NO):
                    wst = wspool.tile([128, 128, 16], F16, name="wst", tag="wst")
                    nc.sync.dma_start(
                        out=wst,
                        in_=bass.AP(tensor=wsd.tensor, offset=o * 128 * 2048,
                                    ap=[[2048, 128], [1, 2048]]))
                    ct = ctpool.tile([128, 2, 8, 16 * 4], F16, name="ct", tag="ct")
                    for qq_ in range(4):
                        for h in range(2):
                            tps = tpsp.tile([128, 128], F16, name="tps", tag="tps")
                            nc.tensor.transpose(
                                tps, dm1[:, o, qq_, h * 128:(h + 1) * 128], idsb)
                            nc.scalar.activation(
                                ct[:, h, :, qq_ * 16:(qq_ + 1) * 16],
                                tps.rearrange("p (a s) -> p a s", a=8), AF.Copy)
                    cu = cupool.tile([128, 2, 8, 8, 64], F16, name="cu", tag="cu")
                    nc.vector.tensor_tensor(
                        out=cu, in0=u_s,
                        in1=ct.unsqueeze(3).broadcast_to([128, 2, 8, 8, 64]),
                        op=ALU.mult)
                    sps = spsp.tile([64, 16], F32, name="sps", tag="sps")
                    for ch in range(128):
                        h, oh, ow = ch // 64, (ch // 8) % 8, ch % 8
                        nc.tensor.matmul(
                            sps, lhsT=cu[:, h, oh, ow, :],
                            rhs=wst[:, ch, :],
                            start=(ch == 0), stop=(ch == 127))
                    nc.scalar.activation(s_all[:, o, :], sps, AF.Copy)
                squash(t)

    import os
    if not os.environ.get("BASS_SKIP_COMPILE"):
        nc.compile()
    return nc


def _host_prep(x, conv_w, conv_b, pcap_w, pcap_b, W):
    x = np.ascontiguousarray(np.asarray(x, np.float32))
    conv_w = np.asarray(conv_w, np.float32)
    conv_b = np.asarray(conv_b, np.float32)
    pcap_w = np.asarray(pcap_w, np.float32)
    pcap_b = np.asarray(pcap_b, np.float32)
    W = np.asarray(W, np.float32)

    w1t = np.ascontiguousarray(
        conv_w.transpose(2, 1, 3, 0).reshape(9, 27, 256)
    ).astype(np.float16)
    cb = np.ascontiguousarray(conv_b.reshape(256, 1))
    w2t = np.ascontiguousarray(
        pcap_w.transpose(1, 2, 3, 0).reshape(2, 128, 81, 256)
    ).astype(np.float16)
    pb = np.ascontiguousarray(pcap_b.reshape(1, 256))

    g = np.zeros((128, 16), np.float32)
    for p in range(128):
        g[p, p // 8] = 1.0
    g = g.astype(np.float16)
    blkmask = (np.arange(128)[:, None] // 16 ==
               np.arange(128)[None, :] // 16).astype(np.float16)
    ident = np.eye(128, dtype=np.float16)
    # repsel[d', (oh,d)] = delta(d==d')
    repsel = (np.arange(16)[:, None] == (np.arange(128)[None, :] % 16)
              ).astype(np.float16)

    W16 = W.astype(np.float16)
    shared = {"w1t": w1t, "cb": cb, "w2t": w2t, "pb": pb, "gmat": g,
              "blkmask": blkmask, "ident": ident, "repsel": repsel}
    in_maps = []
    for c in range(N_CORES):
        n_real = SIZES[c]
        o0 = OFFS[c]
        Wsh = np.zeros((NO, 2048, 16, 8), np.float16)
        Wsh[:n_real] = W16[o0:o0 + n_real]
        # wy[o, oh*16+d, co, k] = W[o, co*8+oh, d, k]
        a = Wsh.reshape(NO, 256, 8, 16, 8)          # [o, co, oh, d, k]
        wy = np.ascontiguousarray(
            a.transpose(0, 2, 3, 1, 4).reshape(NO, 128, 2048)).astype(np.float16)
        # ws[o, p, (h,oh,ow,d)] = W[o, (h*128+p)*8+oh, d, ow]
        b_ = Wsh.reshape(NO, 2, 128, 8, 16, 8)      # [o, h, p, oh, d, ow]
        wsx = b_.transpose(0, 2, 1, 3, 5, 4)        # [o, p, h, oh, ow, d]
        ws = np.ascontiguousarray(wsx.reshape(NO, 128, 2048)).astype(np.float16)
        # ws0[g, p, (j, o*16+d)]: chunk = g*16+j = (h,oh,ow)
        ws0a = wsx.transpose(2, 3, 4, 1, 0, 5).reshape(128, 128, NO * 16)
        ws0 = np.ascontiguousarray(
            ws0a.reshape(8, 16, 128, NO * 16).transpose(0, 2, 1, 3)
            .reshape(8, 128, 16 * NO * 16)).astype(np.float16)
        ob = np.where(np.arange(NO) < n_real, 0.0, -30.0).astype(np.float16)
        obias = np.ascontiguousarray(np.broadcast_to(ob, (128, NO))).copy()

        m = dict(shared)
        m["x_sh"] = np.ascontiguousarray(x[c * B:(c + 1) * B])
        m["wy"] = wy
        m["ws"] = ws
        m["ws0"] = ws0
        m["obias"] = obias.astype(np.float16)
        in_maps.append(m)
    return in_maps


def run(inputs, trace=False, **kw):
    key = "nc"
    if key not in _CACHE:
        _CACHE[key] = _build()
    nc = _CACHE[key]
    in_maps = _host_prep(**inputs)
    res = bass_utils.run_bass_kernel_spmd(
        nc, in_maps, core_ids=list(range(N_CORES)), trace=trace, **kw)
    return res


def kernel(**inputs):
    res = run(inputs)
    v = np.concatenate(
        [res.results[c]["v_out"][:, :SIZES[c], :] for c in range(N_CORES)],
        axis=1)
    return v
